# revision 15
# baseline (speedup 1.0000x reference)
"""Masked multi-head attention (B=32, Lq=Lk=512, H=20, D=20) on 8 TRN2 NeuronCores.

Strategy (v3):
  - Work decomposes into chunk-group jobs (batch, kv-chunk pair, q-range):
    softmax numerator/denominator accumulate linearly over kv chunks and
    queries are independent, so jobs split freely across SPMD rounds; the
    host merges partial (O, sum) outputs and normalizes.
  - Jobs are scheduled into rounds of 8 slots (one per core) with baked
    (NQ, k); q-splitting packs pieces tightly into the baked widths.
  - The host pre-projects Q/K/V (tiny [400,20] weights) straight into the
    PE-ready SBUF layouts (heads at 32-partition offsets for Q/K; per-head
    21-column blocks with a masked ones-column for V, which yields the
    softmax denominator as a free output row).  The device does only the
    quadratic work: per (head-group, chunk) S^T = K_h @ Q_h^T (4 heads at
    32-partition row offsets of the PE), P = exp(S/sqrt(D) - ESHIFT) on the
    ACT engine or on DVE via a Schraudolph exp2 bit-trick (tunable split,
    balancing the two PSUM-draining engines), then O^T += [V_h|1]^T @ P^T
    accumulated over the round's chunks in PSUM.
  - Each head's S^T slice owns full 2KB PSUM banks ([128, 2, 512] tile per
    2 heads): hardware PSUM accumulation groups are bank-granular.
"""

import math

import numpy as np

import concourse.bacc as bacc
import concourse.bass as bass
import concourse.tile as tile
from concourse import mybir
from concourse.bass_utils import run_bass_kernel_spmd

B, LQ, LK = 32, 512, 512
H, D = 20, 20
OUT_DIM = H * D  # 400
N_CORES = 8
NG = 5   # head groups
HPG = 4  # heads per group (partition offsets 0/32/64/96)
KCH = 128
VW = H * 21 + 12  # 432: per-head 20 dims + ones col, padded for 32-wide slices
SCALE = 1.0 / math.sqrt(D)
ESHIFT = 6.0  # exp(s*SCALE - ESHIFT): softmax shift-invariant, keeps P in fp16

F32 = mybir.dt.float32
F16 = mybir.dt.float16
BF16 = mybir.dt.bfloat16
I16 = mybir.dt.int16

# Schraudolph exp2 bit trick (DVE path): bf16 bits of exp(x) ~ round(
#   128*log2(e)*x + (127*128 - C)).  C tunes the mantissa-linearization bias.
EXP_A = 128.0 * math.log2(math.e) * SCALE
EXP_C = 5.0
EXP_B = 16256.0 - 128.0 * math.log2(math.e) * ESHIFT - EXP_C

# Perf knobs
DVE_EXP_8 = 0   # of every 8 exp tiles, this many go to DVE (bit-trick)
TRACE = False
LAST_RESULT = None


# ----------------------------------------------------------------- planning

def _r16(x):
    return max(16, (int(x) + 15) // 16 * 16)


def _plan(q_len, v_len):
    """Decompose into jobs and schedule into SPMD rounds.

    Returns (rounds, assign) where rounds = [(NQ, k)] and
    assign[r][core] = (b, c0, q_lo, q_len_cols) or None."""
    nq = [_r16(min(int(q), LQ)) if int(q) > 0 else 16 for q in q_len]
    kv_eff = [LK if int(v) <= 0 else min(int(v), LK) for v in v_len]
    nkc = [math.ceil(k / KCH) for k in kv_eff]

    pairs, singles = [], []
    for b in range(B):
        k = nkc[b]
        for i in range(k // 2):
            pairs.append((nq[b], b, 2 * i))
        if k % 2:
            singles.append((nq[b], b, k - 1))

    def sched(jobs, k):
        """Split jobs at a cap, rank-match sorted pieces into rounds of 8.
        Search the cap for min predicted cost."""
        if not jobs:
            return [], []
        best = None
        for cap in (512, 448, 384, 320, 288, 256, 224, 192):
            pieces = []
            for (n, b, c0) in jobs:
                q0 = 0
                while n - q0 > cap:
                    pieces.append((cap, b, c0, q0))
                    q0 += cap
                rem = n - q0
                if rem >= 16:
                    pieces.append((rem, b, c0, q0))
            pieces.sort(reverse=True)
            rounds, assign = [], []
            cost = 0.0
            for i in range(0, len(pieces), 8):
                grp = pieces[i:i + 8]
                NQ = grp[0][0]
                rounds.append((NQ, k))
                assign.append([(b, c0, q0, n) for (n, b, c0, q0) in grp]
                              + [None] * (8 - len(grp)))
                # engine-time model (ns): PE streams, ACT/DVE drain pool
                pe = (40 * k * NQ) / 2.4
                drain = (20 * k * NQ + 5 * NQ          # exp + O copy
                         + 10 * k * 280 + 5 * 230) / 2.16
                cost += max(pe, drain) + 600
            if best is None or cost < best[0]:
                best = (cost, rounds, assign)
        return best[1], best[2]

    r2, a2 = sched(pairs, 2)
    r1, a1 = sched(singles, 1)
    both = list(zip(r2 + r1, a2 + a1))
    both.sort(key=lambda x: x[0][0] * x[0][1])
    rounds = [x[0] for x in both]
    assign = [x[1] for x in both]
    return rounds, assign


# ------------------------------------------------------------ device build

def _emit(tc, nc, dr, rounds):
    with (
        tc.tile_pool(name="wpool", bufs=1) as wpool,
        tc.tile_pool(name="seqin", bufs=3) as seqp,
        tc.tile_pool(name="sbp", bufs=4) as sbpp,
        tc.tile_pool(name="sbo", bufs=4) as sbop,
        tc.tile_pool(name="pss", bufs=3, space="PSUM") as pss,
        tc.tile_pool(name="pso", bufs=2, space="PSUM") as pso,
    ):
        eshift = wpool.tile([128, 1], F32, tag="eshift")
        nc.vector.memset(eshift[:], -ESHIFT)

        exp_i = 0
        for r, (NQ, k) in enumerate(rounds):
            nkv = k * KCH
            qp = seqp.tile([128, NG, NQ], F16, tag="qp", name=f"qp{r}")
            kp = seqp.tile([128, NG, nkv], F16, tag="kp", name=f"kp{r}")
            for j in range(HPG):
                nc.sync.dma_start(qp[32 * j:32 * j + D], dr[f"qp{r}"][j])
                nc.sync.dma_start(kp[32 * j:32 * j + D], dr[f"kp{r}"][j])
            vp = seqp.tile([128, k, VW], F16, tag="vp", name=f"vp{r}")
            nc.sync.dma_start(vp[:], dr[f"vp{r}"])

            o = sbop.tile([128, NG, NQ], F32, tag="o", name=f"o{r}")
            for g in range(NG):
                po = pso.tile([128, NQ], F32, tag="po", name=f"po{r}_{g}")
                for c in range(k):
                    pp = []
                    for ip in range(2):
                        ps = pss.tile([128, 2, 512], F32, tag="pss",
                                      name=f"ps{r}_{g}_{c}_{ip}")
                        for jj in range(2):
                            j = 2 * ip + jj
                            nc.tensor.matmul(
                                ps[:, jj, :NQ],
                                kp[32 * j:32 * j + D, g,
                                   c * KCH:(c + 1) * KCH],
                                qp[32 * j:32 * j + D, g, :],
                                start=True, stop=True,
                                tile_position=(32 * j, 0),
                            )
                        pp.append(ps)
                    for ip in range(2):
                        if (exp_i % 8) < DVE_EXP_8:
                            p = sbpp.tile([128, 2, NQ], I16, tag="sbp",
                                          name=f"p{r}_{g}_{c}_{ip}")
                            nc.vector.tensor_scalar(
                                p[:], pp[ip][:, :, :NQ],
                                EXP_A, EXP_B,
                                mybir.AluOpType.mult, mybir.AluOpType.add,
                            )
                            p = p.bitcast(BF16)
                        else:
                            p = sbpp.tile([128, 2, NQ], F16, tag="sbp",
                                          name=f"p{r}_{g}_{c}_{ip}")
                            nc.scalar.activation(
                                p[:], pp[ip][:, :, :NQ],
                                mybir.ActivationFunctionType.Exp,
                                bias=eshift[:], scale=SCALE,
                            )
                        exp_i += 1
                        for jj in range(2):
                            j = 2 * ip + jj
                            h = HPG * g + j
                            nc.tensor.matmul(
                                po[32 * j:32 * j + 32, :],
                                vp[:, c, 21 * h:21 * h + 32],
                                p[:, jj, :],
                                start=(c == 0), stop=(c == k - 1),
                                tile_position=(0, 32 * j),
                                skip_group_check=True,
                            )
                nc.vector.tensor_copy(o[:, g, :], po[:])
            nc.sync.dma_start(dr[f"o{r}"], o[:])


def _build_nc(rounds):
    nc = bacc.Bacc(
        "TRN2",
        target_bir_lowering=False,
        debug=False,
        enable_asserts=False,
        num_devices=N_CORES,
    )
    dr = {}
    for r, (NQ, k) in enumerate(rounds):
        nkv = k * KCH
        dr[f"qp{r}"] = nc.dram_tensor(f"qp{r}", [HPG, D, NG, NQ], F16, kind="ExternalInput").ap()
        dr[f"kp{r}"] = nc.dram_tensor(f"kp{r}", [HPG, D, NG, nkv], F16, kind="ExternalInput").ap()
        dr[f"vp{r}"] = nc.dram_tensor(f"vp{r}", [128, k, VW], F16, kind="ExternalInput").ap()
        dr[f"o{r}"] = nc.dram_tensor(f"o{r}", [128, NG, NQ], F32, kind="ExternalOutput").ap()

    with tile.TileContext(nc) as tc:
        _emit(tc, nc, dr, rounds)
    nc.compile()
    return nc


# ----------------------------------------------------------------- driver

def _project(seq, W, bias):
    """[B, L, 20] @ [400, 20].T + b -> [B, L, 400] fp32."""
    x = seq.reshape(-1, D).astype(np.float32)
    return (x @ W.astype(np.float32).T + bias.astype(np.float32)).reshape(
        seq.shape[0], seq.shape[1], OUT_DIM)


def _prep_core_inputs(core, rounds, assign, V_len, QL, KL, VP):
    m = {}
    for r, (NQ, k) in enumerate(rounds):
        nkv = k * KCH
        qp = np.zeros((HPG, D, NG, NQ), np.float16)
        kp = np.zeros((HPG, D, NG, nkv), np.float16)
        vp = np.zeros((128, k, VW), np.float16)
        job = assign[r][core]
        if job is not None:
            b, c0, q0, qn = job
            qp[:, :, :, :qn] = QL[b][:, :, :, q0:q0 + qn]
            kv_eff = LK if int(V_len[b]) <= 0 else min(int(V_len[b]), LK)
            lo = c0 * KCH
            n = max(0, min(kv_eff - lo, nkv))
            if n > 0:
                kp[:, :, :, :n] = KL[b][:, :, :, lo:lo + n]
                vblock = VP[b][lo:lo + n].reshape(n, H, D)
                for c in range((n + KCH - 1) // KCH):
                    nn = min(KCH, n - c * KCH)
                    tmp = np.zeros((nn, H, 21), np.float16)
                    tmp[:, :, :D] = vblock[c * KCH:c * KCH + nn]
                    tmp[:, :, D] = 1.0
                    vp[:nn, c, :H * 21] = tmp.reshape(nn, H * 21)
        m[f"qp{r}"] = qp
        m[f"kp{r}"] = kp
        m[f"vp{r}"] = vp
    return m


def kernel(**inputs):
    global LAST_RESULT
    Q_seq = np.asarray(inputs["Q_seq"], dtype=np.float32)
    K_seq = np.asarray(inputs["K_seq"], dtype=np.float32)
    V_seq = np.asarray(inputs["V_seq"], dtype=np.float32)
    Q_len = np.asarray(inputs["Q_len"]).reshape(-1).astype(np.int64)
    V_len = np.asarray(inputs["V_len"]).reshape(-1).astype(np.int64)

    rounds, assign = _plan(Q_len, V_len)
    nc = _build_nc(rounds)

    # host projections into PE layouts
    QP = _project(Q_seq, np.asarray(inputs["WQ_w"]), np.asarray(inputs["WQ_b"]))
    KP = _project(K_seq, np.asarray(inputs["WK_w"]), np.asarray(inputs["WK_b"]))
    VP = _project(V_seq, np.asarray(inputs["WV_w"]), np.asarray(inputs["WV_b"]))
    # q/k layout: [b, L, H=(g,j), D] -> [b][j, d, g, col]
    QL = QP.reshape(B, LQ, NG, HPG, D).transpose(0, 3, 4, 2, 1).astype(np.float16)
    KL = KP.reshape(B, LK, NG, HPG, D).transpose(0, 3, 4, 2, 1).astype(np.float16)
    VP = VP.astype(np.float16)

    in_maps = [
        _prep_core_inputs(core, rounds, assign, V_len, QL, KL, VP)
        for core in range(N_CORES)
    ]

    res = run_bass_kernel_spmd(
        nc, in_maps, core_ids=list(range(N_CORES)), trace=TRACE
    )
    LAST_RESULT = res

    num = {}
    den = {}
    for core in range(N_CORES):
        for r, (NQ, k) in enumerate(rounds):
            job = assign[r][core]
            if job is None:
                continue
            b, c0, q0, qn = job
            o = res.results[core][f"o{r}"]  # [128, NG, NQ]
            if b not in num:
                nqb = _r16(min(int(Q_len[b]), LQ)) if int(Q_len[b]) > 0 else 16
                num[b] = np.zeros((H, D, nqb), np.float32)
                den[b] = np.zeros((H, nqb), np.float32)
            oo = o.reshape(HPG, 32, NG, NQ).transpose(2, 0, 1, 3)  # [g,j,32,q]
            oo = oo.reshape(H, 32, NQ)
            num[b][:, :, q0:q0 + qn] += oo[:, :D, :qn]
            den[b][:, q0:q0 + qn] += oo[:, D, :qn]

    out = np.zeros((B, LQ, OUT_DIM), np.float32)
    for b in range(B):
        ql = int(Q_len[b])
        if ql <= 0 or b not in num:
            continue
        ql = min(ql, LQ)
        o = num[b][:, :, :ql] / den[b][:, None, :ql]  # [H, D, ql]
        out[b, :ql, :] = o.transpose(2, 0, 1).reshape(ql, OUT_DIM)
    return out


# revision 20
# speedup vs baseline: 1.0028x; 1.0028x over previous
"""Masked multi-head attention (B=32, Lq=Lk=512, H=20, D=20) on 8 TRN2 NeuronCores.

Strategy (v3):
  - Work decomposes into chunk-group jobs (batch, kv-chunk pair, q-range):
    softmax numerator/denominator accumulate linearly over kv chunks and
    queries are independent, so jobs split freely across SPMD rounds; the
    host merges partial (O, sum) outputs and normalizes.
  - Jobs are scheduled into rounds of 8 slots (one per core) with baked
    (NQ, k); q-splitting packs pieces tightly into the baked widths.
  - The host pre-projects Q/K/V (tiny [400,20] weights) straight into the
    PE-ready SBUF layouts (heads at 32-partition offsets for Q/K; per-head
    21-column blocks with a masked ones-column for V, which yields the
    softmax denominator as a free output row).  The device does only the
    quadratic work: per (head-group, chunk) S^T = K_h @ Q_h^T (4 heads at
    32-partition row offsets of the PE), P = exp(S/sqrt(D) - ESHIFT) on the
    ACT engine or on DVE via a Schraudolph exp2 bit-trick (tunable split,
    balancing the two PSUM-draining engines), then O^T += [V_h|1]^T @ P^T
    accumulated over the round's chunks in PSUM.
  - Each head's S^T slice owns full 2KB PSUM banks ([128, 2, 512] tile per
    2 heads): hardware PSUM accumulation groups are bank-granular.
"""

import math

import numpy as np

import concourse.bacc as bacc
import concourse.bass as bass
import concourse.tile as tile
from concourse import mybir
from concourse.bass_utils import run_bass_kernel_spmd

B, LQ, LK = 32, 512, 512
H, D = 20, 20
OUT_DIM = H * D  # 400
N_CORES = 8
NG = 5   # head groups
HPG = 4  # heads per group (partition offsets 0/32/64/96)
KCH = 128
VW = H * 21 + 12  # 432: per-head 20 dims + ones col, padded for 32-wide slices
SCALE = 1.0 / math.sqrt(D)
ESHIFT = 6.0  # exp(s*SCALE - ESHIFT): softmax shift-invariant, keeps P in fp16

F32 = mybir.dt.float32
F16 = mybir.dt.float16
F8 = mybir.dt.float8e4
BF16 = mybir.dt.bfloat16
I16 = mybir.dt.int16

# Schraudolph exp2 bit trick (DVE path): bf16 bits of exp(x) ~ round(
#   128*log2(e)*x + (127*128 - C)).  C tunes the mantissa-linearization bias.
EXP_A = 128.0 * math.log2(math.e) * SCALE
EXP_C = 5.0
EXP_B = 16256.0 - 128.0 * math.log2(math.e) * ESHIFT - EXP_C

# Perf knobs
DVE_EXP_8 = 0   # of every 8 exp tiles, this many go to DVE (bit-trick)
FP8_S = False   # fp8e4m3 Q/K + DoubleRow perf mode for the S^T matmuls
TRACE = False
LAST_RESULT = None


# ----------------------------------------------------------------- planning

def _r16(x):
    return max(16, (int(x) + 15) // 16 * 16)


def _plan(q_len, v_len):
    """Decompose into jobs and schedule into SPMD rounds.

    Returns (rounds, assign) where rounds = [(NQ, k)] and
    assign[r][core] = (b, c0, q_lo, q_len_cols) or None."""
    nq = [_r16(min(int(q), LQ)) if int(q) > 0 else 16 for q in q_len]
    kv_eff = [LK if int(v) <= 0 else min(int(v), LK) for v in v_len]
    nkc = [math.ceil(k / KCH) for k in kv_eff]

    pairs, singles = [], []
    for b in range(B):
        k = nkc[b]
        for i in range(k // 2):
            pairs.append((nq[b], b, 2 * i))
        if k % 2:
            singles.append((nq[b], b, k - 1))

    def sched(jobs, k):
        """Split jobs at a cap, rank-match sorted pieces into rounds of 8.
        Search the cap for min predicted cost."""
        if not jobs:
            return [], []
        best = None
        for cap in (512, 448, 384, 320, 288, 256, 224, 192):
            pieces = []
            for (n, b, c0) in jobs:
                q0 = 0
                while n - q0 > cap:
                    pieces.append((cap, b, c0, q0))
                    q0 += cap
                rem = n - q0
                if rem >= 16:
                    pieces.append((rem, b, c0, q0))
            pieces.sort(reverse=True)
            rounds, assign = [], []
            cost = 0.0
            for i in range(0, len(pieces), 8):
                grp = pieces[i:i + 8]
                NQ = grp[0][0]
                rounds.append((NQ, k))
                assign.append([(b, c0, q0, n) for (n, b, c0, q0) in grp]
                              + [None] * (8 - len(grp)))
                # engine-time model (ns): PE streams, ACT/DVE drain pool
                pe = (40 * k * NQ) / 2.4
                drain = (20 * k * NQ + 5 * NQ          # exp + O copy
                         + 10 * k * 280 + 5 * 230) / 2.16
                cost += max(pe, drain) + 600
            if best is None or cost < best[0]:
                best = (cost, rounds, assign)
        return best[1], best[2]

    r2, a2 = sched(pairs, 2)
    r1, a1 = sched(singles, 1)
    rounds = r2 + r1
    assign = a2 + a1
    return rounds, assign


# ------------------------------------------------------------ device build

def _emit(tc, nc, dr, rounds):
    with (
        tc.tile_pool(name="wpool", bufs=1) as wpool,
        tc.tile_pool(name="seqin", bufs=3) as seqp,
        tc.tile_pool(name="sbp", bufs=4) as sbpp,
        tc.tile_pool(name="sbo", bufs=4) as sbop,
        tc.tile_pool(name="pss", bufs=3, space="PSUM") as pss,
        tc.tile_pool(name="pso", bufs=2, space="PSUM") as pso,
    ):
        eshift = wpool.tile([128, 1], F32, tag="eshift")
        nc.vector.memset(eshift[:], -ESHIFT)

        exp_i = 0
        for r, (NQ, k) in enumerate(rounds):
            nkv = k * KCH
            if FP8_S:
                qp = seqp.tile([128, NG, 2, NQ], F8, tag="qp", name=f"qp{r}")
                kp = seqp.tile([128, NG, 2, nkv], F8, tag="kp", name=f"kp{r}")
            else:
                qp = seqp.tile([128, NG, NQ], F16, tag="qp", name=f"qp{r}")
                kp = seqp.tile([128, NG, nkv], F16, tag="kp", name=f"kp{r}")
            vp = seqp.tile([128, k, VW], F16, tag="vp", name=f"vp{r}")
            for g in range(NG):
                nc.gpsimd.dma_start(qp[:, g], dr[f"qp{r}"][:, g])
                nc.gpsimd.dma_start(kp[:, g], dr[f"kp{r}"][:, g])
            nc.gpsimd.dma_start(vp[:], dr[f"vp{r}"])

            o = sbop.tile([128, NG, NQ], F32, tag="o", name=f"o{r}")
            for g in range(NG):
                po = pso.tile([128, NQ], F32, tag="po", name=f"po{r}_{g}")
                for c in range(k):
                    pp = []
                    for ip in range(2):
                        ps = pss.tile([128, 2, 512], F32, tag="pss",
                                      name=f"ps{r}_{g}_{c}_{ip}")
                        for jj in range(2):
                            j = 2 * ip + jj
                            if FP8_S:
                                nc.tensor.matmul(
                                    ps[:, jj, :NQ],
                                    kp[32 * j:32 * j + D // 2, g, :,
                                       c * KCH:(c + 1) * KCH],
                                    qp[32 * j:32 * j + D // 2, g, :, :],
                                    start=True, stop=True,
                                    perf_mode=mybir.MatmulPerfMode.DoubleRow,
                                    tile_position=(32 * j, 0),
                                )
                            else:
                                nc.tensor.matmul(
                                    ps[:, jj, :NQ],
                                    kp[32 * j:32 * j + D, g,
                                       c * KCH:(c + 1) * KCH],
                                    qp[32 * j:32 * j + D, g, :],
                                    start=True, stop=True,
                                    tile_position=(32 * j, 0),
                                )
                        pp.append(ps)
                    for ip in range(2):
                        if (exp_i % 8) < DVE_EXP_8:
                            p = sbpp.tile([128, 2, NQ], I16, tag="sbp",
                                          name=f"p{r}_{g}_{c}_{ip}")
                            nc.vector.tensor_scalar(
                                p[:], pp[ip][:, :, :NQ],
                                EXP_A, EXP_B,
                                mybir.AluOpType.mult, mybir.AluOpType.add,
                            )
                            p = p.bitcast(BF16)
                        else:
                            p = sbpp.tile([128, 2, NQ], F16, tag="sbp",
                                          name=f"p{r}_{g}_{c}_{ip}")
                            nc.scalar.activation(
                                p[:], pp[ip][:, :, :NQ],
                                mybir.ActivationFunctionType.Exp,
                                bias=eshift[:], scale=SCALE,
                            )
                        exp_i += 1
                        for jj in range(2):
                            j = 2 * ip + jj
                            h = HPG * g + j
                            nc.tensor.matmul(
                                po[32 * j:32 * j + 32, :],
                                vp[:, c, 21 * h:21 * h + 32],
                                p[:, jj, :],
                                start=(c == 0), stop=(c == k - 1),
                                tile_position=(0, 32 * j),
                                skip_group_check=True,
                            )
                nc.vector.tensor_copy(o[:, g, :], po[:])
                nc.gpsimd.dma_start(dr[f"o{r}"][:, g, :], o[:, g, :])


def _build_nc(rounds):
    nc = bacc.Bacc(
        "TRN2",
        target_bir_lowering=False,
        debug=False,
        enable_asserts=False,
        num_devices=N_CORES,
    )
    dr = {}
    for r, (NQ, k) in enumerate(rounds):
        nkv = k * KCH
        if FP8_S:
            dr[f"qp{r}"] = nc.dram_tensor(f"qp{r}", [128, NG, 2, NQ], F8, kind="ExternalInput").ap()
            dr[f"kp{r}"] = nc.dram_tensor(f"kp{r}", [128, NG, 2, nkv], F8, kind="ExternalInput").ap()
        else:
            dr[f"qp{r}"] = nc.dram_tensor(f"qp{r}", [128, NG, NQ], F16, kind="ExternalInput").ap()
            dr[f"kp{r}"] = nc.dram_tensor(f"kp{r}", [128, NG, nkv], F16, kind="ExternalInput").ap()
        dr[f"vp{r}"] = nc.dram_tensor(f"vp{r}", [128, k, VW], F16, kind="ExternalInput").ap()
        dr[f"o{r}"] = nc.dram_tensor(f"o{r}", [128, NG, NQ], F32, kind="ExternalOutput").ap()

    with tile.TileContext(nc) as tc:
        _emit(tc, nc, dr, rounds)
    nc.compile()
    return nc


# ----------------------------------------------------------------- driver

def _project(seq, W, bias):
    """[B, L, 20] @ [400, 20].T + b -> [B, L, 400] fp32."""
    x = seq.reshape(-1, D).astype(np.float32)
    return (x @ W.astype(np.float32).T + bias.astype(np.float32)).reshape(
        seq.shape[0], seq.shape[1], OUT_DIM)


def _prep_core_inputs(core, rounds, assign, V_len, QL, KL, VP):
    m = {}
    for r, (NQ, k) in enumerate(rounds):
        nkv = k * KCH
        if FP8_S:
            qp = np.zeros((128, NG, 2, NQ), QL.dtype)
            kp = np.zeros((128, NG, 2, nkv), QL.dtype)
        else:
            qp = np.zeros((128, NG, NQ), np.float16)
            kp = np.zeros((128, NG, nkv), np.float16)
        vp = np.zeros((128, k, VW), np.float16)
        job = assign[r][core]
        if job is not None:
            b, c0, q0, qn = job
            if FP8_S:
                qp.reshape(HPG, 32, NG, 2, NQ)[:, :D // 2, :, :, :qn] = \
                    QL[b][:, :, :, :, q0:q0 + qn]
            else:
                qp.reshape(HPG, 32, NG, NQ)[:, :D, :, :qn] = \
                    QL[b][:, :, :, q0:q0 + qn]
            kv_eff = LK if int(V_len[b]) <= 0 else min(int(V_len[b]), LK)
            lo = c0 * KCH
            n = max(0, min(kv_eff - lo, nkv))
            if n > 0:
                if FP8_S:
                    kp.reshape(HPG, 32, NG, 2, nkv)[:, :D // 2, :, :, :n] = \
                        KL[b][:, :, :, :, lo:lo + n]
                else:
                    kp.reshape(HPG, 32, NG, nkv)[:, :D, :, :n] = \
                        KL[b][:, :, :, lo:lo + n]
                vblock = VP[b][lo:lo + n].reshape(n, H, D)
                for c in range((n + KCH - 1) // KCH):
                    nn = min(KCH, n - c * KCH)
                    tmp = np.zeros((nn, H, 21), np.float16)
                    tmp[:, :, :D] = vblock[c * KCH:c * KCH + nn]
                    tmp[:, :, D] = 1.0
                    vp[:nn, c, :H * 21] = tmp.reshape(nn, H * 21)
        m[f"qp{r}"] = qp
        m[f"kp{r}"] = kp
        m[f"vp{r}"] = vp
    return m


def _layouts(inputs):
    QP = _project(np.asarray(inputs["Q_seq"], np.float32), np.asarray(inputs["WQ_w"]), np.asarray(inputs["WQ_b"]))
    KP = _project(np.asarray(inputs["K_seq"], np.float32), np.asarray(inputs["WK_w"]), np.asarray(inputs["WK_b"]))
    VP = _project(np.asarray(inputs["V_seq"], np.float32), np.asarray(inputs["WV_w"]), np.asarray(inputs["WV_b"]))
    if FP8_S:
        import ml_dtypes
        f8 = ml_dtypes.float8_e4m3fn
        QL = QP.reshape(B, LQ, NG, HPG, 2, D // 2).transpose(
            0, 3, 5, 2, 4, 1).astype(f8)
        KL = KP.reshape(B, LK, NG, HPG, 2, D // 2).transpose(
            0, 3, 5, 2, 4, 1).astype(f8)
    else:
        QL = QP.reshape(B, LQ, NG, HPG, D).transpose(0, 3, 4, 2, 1).astype(np.float16)
        KL = KP.reshape(B, LK, NG, HPG, D).transpose(0, 3, 4, 2, 1).astype(np.float16)
    return QL, KL, VP.astype(np.float16)


def kernel(**inputs):
    global LAST_RESULT
    Q_seq = np.asarray(inputs["Q_seq"], dtype=np.float32)
    K_seq = np.asarray(inputs["K_seq"], dtype=np.float32)
    V_seq = np.asarray(inputs["V_seq"], dtype=np.float32)
    Q_len = np.asarray(inputs["Q_len"]).reshape(-1).astype(np.int64)
    V_len = np.asarray(inputs["V_len"]).reshape(-1).astype(np.int64)

    rounds, assign = _plan(Q_len, V_len)
    nc = _build_nc(rounds)

    QL, KL, VP = _layouts(inputs)

    in_maps = [
        _prep_core_inputs(core, rounds, assign, V_len, QL, KL, VP)
        for core in range(N_CORES)
    ]

    res = run_bass_kernel_spmd(
        nc, in_maps, core_ids=list(range(N_CORES)), trace=TRACE
    )
    LAST_RESULT = res

    num = {}
    den = {}
    for core in range(N_CORES):
        for r, (NQ, k) in enumerate(rounds):
            job = assign[r][core]
            if job is None:
                continue
            b, c0, q0, qn = job
            o = res.results[core][f"o{r}"]  # [128, NG, NQ]
            if b not in num:
                nqb = _r16(min(int(Q_len[b]), LQ)) if int(Q_len[b]) > 0 else 16
                num[b] = np.zeros((H, D, nqb), np.float32)
                den[b] = np.zeros((H, nqb), np.float32)
            oo = o.reshape(HPG, 32, NG, NQ).transpose(2, 0, 1, 3)  # [g,j,32,q]
            oo = oo.reshape(H, 32, NQ)
            num[b][:, :, q0:q0 + qn] += oo[:, :D, :qn]
            den[b][:, q0:q0 + qn] += oo[:, D, :qn]

    out = np.zeros((B, LQ, OUT_DIM), np.float32)
    for b in range(B):
        ql = int(Q_len[b])
        if ql <= 0 or b not in num:
            continue
        ql = min(ql, LQ)
        o = num[b][:, :, :ql] / den[b][:, None, :ql]  # [H, D, ql]
        out[b, :ql, :] = o.transpose(2, 0, 1).reshape(ql, OUT_DIM)
    return out


# revision 22
# speedup vs baseline: 1.0770x; 1.0739x over previous
"""Masked multi-head attention (B=32, Lq=Lk=512, H=20, D=20) on 8 TRN2 NeuronCores.

Strategy (v3):
  - Work decomposes into chunk-group jobs (batch, kv-chunk pair, q-range):
    softmax numerator/denominator accumulate linearly over kv chunks and
    queries are independent, so jobs split freely across SPMD rounds; the
    host merges partial (O, sum) outputs and normalizes.
  - Jobs are scheduled into rounds of 8 slots (one per core) with baked
    (NQ, k); q-splitting packs pieces tightly into the baked widths.
  - The host pre-projects Q/K/V (tiny [400,20] weights) straight into the
    PE-ready SBUF layouts (heads at 32-partition offsets for Q/K; per-head
    21-column blocks with a masked ones-column for V, which yields the
    softmax denominator as a free output row).  The device does only the
    quadratic work: per (head-group, chunk) S^T = K_h @ Q_h^T (4 heads at
    32-partition row offsets of the PE), P = exp(S/sqrt(D) - ESHIFT) on the
    ACT engine or on DVE via a Schraudolph exp2 bit-trick (tunable split,
    balancing the two PSUM-draining engines), then O^T += [V_h|1]^T @ P^T
    accumulated over the round's chunks in PSUM.
  - Each head's S^T slice owns full 2KB PSUM banks ([128, 2, 512] tile per
    2 heads): hardware PSUM accumulation groups are bank-granular.
"""

import math

import numpy as np

import concourse.bacc as bacc
import concourse.bass as bass
import concourse.tile as tile
from concourse import mybir
from concourse.bass_utils import run_bass_kernel_spmd

B, LQ, LK = 32, 512, 512
H, D = 20, 20
OUT_DIM = H * D  # 400
N_CORES = 8
NG = 5   # head groups
HPG = 4  # heads per group (partition offsets 0/32/64/96)
KCH = 128
VW = H * 21 + 12  # 432: per-head 20 dims + ones col, padded for 32-wide slices
SCALE = 1.0 / math.sqrt(D)
ESHIFT = 6.0  # exp(s*SCALE - ESHIFT): softmax shift-invariant, keeps P in fp16

F32 = mybir.dt.float32
F16 = mybir.dt.float16
F8 = mybir.dt.float8e4
BF16 = mybir.dt.bfloat16
I16 = mybir.dt.int16

# Schraudolph exp2 bit trick (DVE path): bf16 bits of exp(x) ~ round(
#   128*log2(e)*x + (127*128 - C)).  C tunes the mantissa-linearization bias.
EXP_A = 128.0 * math.log2(math.e) * SCALE
EXP_C = 5.0
EXP_B = 16256.0 - 128.0 * math.log2(math.e) * ESHIFT - EXP_C

# Perf knobs
DVE_EXP_8 = 0   # of every 8 exp tiles, this many go to DVE (bit-trick)
FP8_S = False   # fp8e4m3 Q/K + DoubleRow perf mode for the S^T matmuls
TRACE = False
LAST_RESULT = None


# ----------------------------------------------------------------- planning

def _r16(x):
    return max(16, (int(x) + 15) // 16 * 16)


def _plan(q_len, v_len):
    """Decompose into jobs and schedule into SPMD rounds.

    Returns (rounds, assign) where rounds = [(NQ, k)] and
    assign[r][core] = (b, c0, q_lo, q_len_cols) or None."""
    nq = [_r16(min(int(q), LQ)) if int(q) > 0 else 16 for q in q_len]
    kv_eff = [LK if int(v) <= 0 else min(int(v), LK) for v in v_len]
    nkc = [math.ceil(k / KCH) for k in kv_eff]

    pairs, singles = [], []
    for b in range(B):
        k = nkc[b]
        for i in range(k // 2):
            pairs.append((nq[b], b, 2 * i))
        if k % 2:
            singles.append((nq[b], b, k - 1))

    def sched(jobs, k):
        """Split jobs at a cap, rank-match sorted pieces into rounds of 8.
        Search the cap for min predicted cost."""
        if not jobs:
            return [], []
        best = None
        for cap in (512, 448, 384, 320, 288, 256, 224, 192):
            pieces = []
            for (n, b, c0) in jobs:
                q0 = 0
                while n - q0 > cap:
                    pieces.append((cap, b, c0, q0))
                    q0 += cap
                rem = n - q0
                if rem >= 16:
                    pieces.append((rem, b, c0, q0))
            pieces.sort(reverse=True)
            rounds, assign = [], []
            cost = 0.0
            for i in range(0, len(pieces), 8):
                grp = pieces[i:i + 8]
                NQ = grp[0][0]
                rounds.append((NQ, k))
                assign.append([(b, c0, q0, n) for (n, b, c0, q0) in grp]
                              + [None] * (8 - len(grp)))
                # engine-time model (ns): PE streams, ACT/DVE drain pool
                pe = (40 * k * NQ) / 2.4
                drain = (20 * k * NQ + 5 * NQ          # exp + O copy
                         + 10 * k * 280 + 5 * 230) / 2.16
                cost += max(pe, drain) + 600
            if best is None or cost < best[0]:
                best = (cost, rounds, assign)
        return best[1], best[2]

    r2, a2 = sched(pairs, 2)
    r1, a1 = sched(singles, 1)
    rounds = r2 + r1
    assign = a2 + a1
    if len(rounds) > 1:
        rounds = [rounds[-1]] + rounds[:-1]
        assign = [assign[-1]] + assign[:-1]
    return rounds, assign


# ------------------------------------------------------------ device build

def _emit(tc, nc, dr, rounds):
    with (
        tc.tile_pool(name="wpool", bufs=1) as wpool,
        tc.tile_pool(name="seqin", bufs=3) as seqp,
        tc.tile_pool(name="sbp", bufs=4) as sbpp,
        tc.tile_pool(name="sbo", bufs=4) as sbop,
        tc.tile_pool(name="pss", bufs=3, space="PSUM") as pss,
        tc.tile_pool(name="pso", bufs=2, space="PSUM") as pso,
    ):
        eshift = wpool.tile([128, 1], F32, tag="eshift")
        nc.vector.memset(eshift[:], -ESHIFT)

        exp_i = 0
        for r, (NQ, k) in enumerate(rounds):
            nkv = k * KCH
            if FP8_S:
                qp = seqp.tile([128, NG, 2, NQ], F8, tag="qp", name=f"qp{r}")
                kp = seqp.tile([128, NG, 2, nkv], F8, tag="kp", name=f"kp{r}")
            else:
                qp = seqp.tile([128, NG, NQ], F16, tag="qp", name=f"qp{r}")
                kp = seqp.tile([128, NG, nkv], F16, tag="kp", name=f"kp{r}")
            vp = seqp.tile([128, k, VW], F16, tag="vp", name=f"vp{r}")
            nc.sync.dma_start(qp[:], dr[f"qp{r}"])
            nc.sync.dma_start(kp[:], dr[f"kp{r}"])
            nc.sync.dma_start(vp[:], dr[f"vp{r}"])

            o = sbop.tile([128, NG, NQ], F32, tag="o", name=f"o{r}")
            for g in range(NG):
                po = pso.tile([128, NQ], F32, tag="po", name=f"po{r}_{g}")
                for c in range(k):
                    pp = []
                    for ip in range(2):
                        ps = pss.tile([128, 2, 512], F32, tag="pss",
                                      name=f"ps{r}_{g}_{c}_{ip}")
                        for jj in range(2):
                            j = 2 * ip + jj
                            if FP8_S:
                                nc.tensor.matmul(
                                    ps[:, jj, :NQ],
                                    kp[32 * j:32 * j + D // 2, g, :,
                                       c * KCH:(c + 1) * KCH],
                                    qp[32 * j:32 * j + D // 2, g, :, :],
                                    start=True, stop=True,
                                    perf_mode=mybir.MatmulPerfMode.DoubleRow,
                                    tile_position=(32 * j, 0),
                                )
                            else:
                                nc.tensor.matmul(
                                    ps[:, jj, :NQ],
                                    kp[32 * j:32 * j + D, g,
                                       c * KCH:(c + 1) * KCH],
                                    qp[32 * j:32 * j + D, g, :],
                                    start=True, stop=True,
                                    tile_position=(32 * j, 0),
                                )
                        pp.append(ps)
                    for ip in range(2):
                        if (exp_i % 8) < DVE_EXP_8:
                            p = sbpp.tile([128, 2, NQ], I16, tag="sbp",
                                          name=f"p{r}_{g}_{c}_{ip}")
                            nc.vector.tensor_scalar(
                                p[:], pp[ip][:, :, :NQ],
                                EXP_A, EXP_B,
                                mybir.AluOpType.mult, mybir.AluOpType.add,
                            )
                            p = p.bitcast(BF16)
                        else:
                            p = sbpp.tile([128, 2, NQ], F16, tag="sbp",
                                          name=f"p{r}_{g}_{c}_{ip}")
                            nc.scalar.activation(
                                p[:], pp[ip][:, :, :NQ],
                                mybir.ActivationFunctionType.Exp,
                                bias=eshift[:], scale=SCALE,
                            )
                        exp_i += 1
                        for jj in range(2):
                            j = 2 * ip + jj
                            h = HPG * g + j
                            nc.tensor.matmul(
                                po[32 * j:32 * j + 32, :],
                                vp[:, c, 21 * h:21 * h + 32],
                                p[:, jj, :],
                                start=(c == 0), stop=(c == k - 1),
                                tile_position=(0, 32 * j),
                                skip_group_check=True,
                            )
                nc.vector.tensor_copy(o[:, g, :], po[:])
            nc.sync.dma_start(dr[f"o{r}"], o[:])


def _build_nc(rounds):
    nc = bacc.Bacc(
        "TRN2",
        target_bir_lowering=False,
        debug=False,
        enable_asserts=False,
        num_devices=N_CORES,
    )
    dr = {}
    for r, (NQ, k) in enumerate(rounds):
        nkv = k * KCH
        if FP8_S:
            dr[f"qp{r}"] = nc.dram_tensor(f"qp{r}", [128, NG, 2, NQ], F8, kind="ExternalInput").ap()
            dr[f"kp{r}"] = nc.dram_tensor(f"kp{r}", [128, NG, 2, nkv], F8, kind="ExternalInput").ap()
        else:
            dr[f"qp{r}"] = nc.dram_tensor(f"qp{r}", [128, NG, NQ], F16, kind="ExternalInput").ap()
            dr[f"kp{r}"] = nc.dram_tensor(f"kp{r}", [128, NG, nkv], F16, kind="ExternalInput").ap()
        dr[f"vp{r}"] = nc.dram_tensor(f"vp{r}", [128, k, VW], F16, kind="ExternalInput").ap()
        dr[f"o{r}"] = nc.dram_tensor(f"o{r}", [128, NG, NQ], F32, kind="ExternalOutput").ap()

    with tile.TileContext(nc) as tc:
        _emit(tc, nc, dr, rounds)
    nc.compile()
    return nc


# ----------------------------------------------------------------- driver

def _project(seq, W, bias):
    """[B, L, 20] @ [400, 20].T + b -> [B, L, 400] fp32."""
    x = seq.reshape(-1, D).astype(np.float32)
    return (x @ W.astype(np.float32).T + bias.astype(np.float32)).reshape(
        seq.shape[0], seq.shape[1], OUT_DIM)


def _prep_core_inputs(core, rounds, assign, V_len, QL, KL, VP):
    m = {}
    for r, (NQ, k) in enumerate(rounds):
        nkv = k * KCH
        if FP8_S:
            qp = np.zeros((128, NG, 2, NQ), QL.dtype)
            kp = np.zeros((128, NG, 2, nkv), QL.dtype)
        else:
            qp = np.zeros((128, NG, NQ), np.float16)
            kp = np.zeros((128, NG, nkv), np.float16)
        vp = np.zeros((128, k, VW), np.float16)
        job = assign[r][core]
        if job is not None:
            b, c0, q0, qn = job
            if FP8_S:
                qp.reshape(HPG, 32, NG, 2, NQ)[:, :D // 2, :, :, :qn] = \
                    QL[b][:, :, :, :, q0:q0 + qn]
            else:
                qp.reshape(HPG, 32, NG, NQ)[:, :D, :, :qn] = \
                    QL[b][:, :, :, q0:q0 + qn]
            kv_eff = LK if int(V_len[b]) <= 0 else min(int(V_len[b]), LK)
            lo = c0 * KCH
            n = max(0, min(kv_eff - lo, nkv))
            if n > 0:
                if FP8_S:
                    kp.reshape(HPG, 32, NG, 2, nkv)[:, :D // 2, :, :, :n] = \
                        KL[b][:, :, :, :, lo:lo + n]
                else:
                    kp.reshape(HPG, 32, NG, nkv)[:, :D, :, :n] = \
                        KL[b][:, :, :, lo:lo + n]
                vblock = VP[b][lo:lo + n].reshape(n, H, D)
                for c in range((n + KCH - 1) // KCH):
                    nn = min(KCH, n - c * KCH)
                    tmp = np.zeros((nn, H, 21), np.float16)
                    tmp[:, :, :D] = vblock[c * KCH:c * KCH + nn]
                    tmp[:, :, D] = 1.0
                    vp[:nn, c, :H * 21] = tmp.reshape(nn, H * 21)
        m[f"qp{r}"] = qp
        m[f"kp{r}"] = kp
        m[f"vp{r}"] = vp
    return m


def _layouts(inputs):
    QP = _project(np.asarray(inputs["Q_seq"], np.float32), np.asarray(inputs["WQ_w"]), np.asarray(inputs["WQ_b"]))
    KP = _project(np.asarray(inputs["K_seq"], np.float32), np.asarray(inputs["WK_w"]), np.asarray(inputs["WK_b"]))
    VP = _project(np.asarray(inputs["V_seq"], np.float32), np.asarray(inputs["WV_w"]), np.asarray(inputs["WV_b"]))
    if FP8_S:
        import ml_dtypes
        f8 = ml_dtypes.float8_e4m3fn
        QL = QP.reshape(B, LQ, NG, HPG, 2, D // 2).transpose(
            0, 3, 5, 2, 4, 1).astype(f8)
        KL = KP.reshape(B, LK, NG, HPG, 2, D // 2).transpose(
            0, 3, 5, 2, 4, 1).astype(f8)
    else:
        QL = QP.reshape(B, LQ, NG, HPG, D).transpose(0, 3, 4, 2, 1).astype(np.float16)
        KL = KP.reshape(B, LK, NG, HPG, D).transpose(0, 3, 4, 2, 1).astype(np.float16)
    return QL, KL, VP.astype(np.float16)


def kernel(**inputs):
    global LAST_RESULT
    Q_seq = np.asarray(inputs["Q_seq"], dtype=np.float32)
    K_seq = np.asarray(inputs["K_seq"], dtype=np.float32)
    V_seq = np.asarray(inputs["V_seq"], dtype=np.float32)
    Q_len = np.asarray(inputs["Q_len"]).reshape(-1).astype(np.int64)
    V_len = np.asarray(inputs["V_len"]).reshape(-1).astype(np.int64)

    rounds, assign = _plan(Q_len, V_len)
    nc = _build_nc(rounds)

    QL, KL, VP = _layouts(inputs)

    in_maps = [
        _prep_core_inputs(core, rounds, assign, V_len, QL, KL, VP)
        for core in range(N_CORES)
    ]

    res = run_bass_kernel_spmd(
        nc, in_maps, core_ids=list(range(N_CORES)), trace=TRACE
    )
    LAST_RESULT = res

    num = {}
    den = {}
    for core in range(N_CORES):
        for r, (NQ, k) in enumerate(rounds):
            job = assign[r][core]
            if job is None:
                continue
            b, c0, q0, qn = job
            o = res.results[core][f"o{r}"]  # [128, NG, NQ]
            if b not in num:
                nqb = _r16(min(int(Q_len[b]), LQ)) if int(Q_len[b]) > 0 else 16
                num[b] = np.zeros((H, D, nqb), np.float32)
                den[b] = np.zeros((H, nqb), np.float32)
            oo = o.reshape(HPG, 32, NG, NQ).transpose(2, 0, 1, 3)  # [g,j,32,q]
            oo = oo.reshape(H, 32, NQ)
            num[b][:, :, q0:q0 + qn] += oo[:, :D, :qn]
            den[b][:, q0:q0 + qn] += oo[:, D, :qn]

    out = np.zeros((B, LQ, OUT_DIM), np.float32)
    for b in range(B):
        ql = int(Q_len[b])
        if ql <= 0 or b not in num:
            continue
        ql = min(ql, LQ)
        o = num[b][:, :, :ql] / den[b][:, None, :ql]  # [H, D, ql]
        out[b, :ql, :] = o.transpose(2, 0, 1).reshape(ql, OUT_DIM)
    return out
